# revision 31
# baseline (speedup 1.0000x reference)
"""Trainium2 Bass kernel for the Human3.6M pose postprocess
(spherical->xyz conversion + kinematic-tree accumulation).

Self-contained: hardcodes the problem shapes
  observed_pose (4096, 16, 96) f32, pred_pose (4096, 64, 66) f32
and shards the batch dim across 8 NeuronCores (pure data parallel,
512 examples per core).

Per-core design (bt-major layout, no transposes, no TensorE):
  - partition p <-> 32 consecutive time rows of one batch example;
    free dim holds (time reps) x channels.  All DMA is >=4KB contiguous
    per partition.
  - trig via half-angle identities, so no range reduction is needed
    (|x/2| <= 2.85 < pi fits the Sin LUT):
      s = Sin(x/2), c = cos(x/2) = Sin(pi/2 - |x|/2)
      sin x = 2sc,  cos x = 1 - 2s^2
    with all 2x/4x factors folded into a 4r copy and affine biases.
  - the spherical->xyz products are written by run-split TRIPLE muls
    ([rsin4|4r|rsin4] x [ct~|cp~|st~] -> one DVE op per channel run covers
    all three components) DIRECTLY into the scan work buffer W's
    output-channel slots (no assembly copies); correction slots are
    reduced from W itself.
  - ONE gated tensor_tensor_scan per tile (3 components fused) computes
    the entire 22-edge kinematic tree walk in natural output-channel
    order:  state = gate*state + W;  gate=0 at root channels {0,1,6,11}
    reseeds state from the last observed frame, zero-offset slots
    reproduce the IGNORE copies, and two correction slots (ch16, ch24)
    rewind the state to x[13] across tree branches.
Output leaves the device component-major [nb, 3, 64, 32]; the host
transposes back to (B, T, 96).

Measured on trn2 (8 cores via axon): HW exec ~151 us/core,
relative error vs the jax reference: 1.1e-7.
"""

import math
import sys

for _p in ("/opt/trn_rl_repo", "/root/.axon_site/_ro/trn_rl_repo"):
    if _p not in sys.path:
        sys.path.insert(0, _p)

import numpy as np

PI = math.pi
BIGC = 1.5 * 2**23  # fp32 round-to-nearest-integer constant
T = 64   # time steps = reps per partition
P = 128  # partitions per tile

N_CORES = 8
B = 4096
NB = B // N_CORES  # batches per core
KERNEL_R = 32      # rows (time steps) per partition

# child-joint order of CONNECT (k index) -> contiguous runs in output-channel
# space: (k_start, ch_start, length)
ASSEM_RUNS = [
    (0, 12, 4),   # k0..3   -> ch12..15  (spine 12,13,14,15)
    (4, 25, 3),   # k4..6   -> ch25..27  (arm 25,26,27)
    (7, 29, 2),   # k7..8   -> ch29..30  (arm 29,30)
    (9, 17, 3),   # k9..11  -> ch17..19  (arm 17,18,19)
    (12, 21, 2),  # k12..13 -> ch21..22  (arm 21,22)
    (14, 2, 4),   # k14..17 -> ch2..5    (leg 2,3,4,5)
    (18, 7, 4),   # k18..21 -> ch7..10   (leg 7,8,9,10)
]

# equal-length runs pair-merged (second run reached by one extra AP dim)
RUN_GROUPS = [
    [(0, 12, 4)],
    [(14, 2, 4), (18, 7, 4)],
    [(4, 25, 3), (9, 17, 3)],
    [(7, 29, 2), (12, 21, 2)],
]


def build_kernel(nc, n_b: int, r: int = 32):
    """Build the postprocess kernel for n_b batch examples on one core."""
    import concourse.tile as tile
    from concourse import mybir

    f32 = mybir.dt.float32
    pred = nc.dram_tensor("pred", [n_b * T, 66], f32, kind="ExternalInput")
    # obs rows pre-duplicated on the host: row j <-> partition j of a tile
    # (each batch example spans 64//r partitions)
    obs = nc.dram_tensor("obs", [n_b * (T // r), 96], f32, kind="ExternalInput")
    out = nc.dram_tensor("out", [n_b, 3 * T * 32], f32, kind="ExternalOutput")

    with tile.TileContext(nc) as tc:
        build_tile_kernel(tc, pred, obs, out, n_b, r)
    return nc


def build_tile_kernel(tc, pred, obs, out, n_b: int, r: int):
    import concourse.bass as bass
    from concourse import mybir

    f32 = mybir.dt.float32
    ALU = mybir.AluOpType
    ACTF = mybir.ActivationFunctionType
    nc = tc.nc
    pp = min(P, n_b * T // r)
    nt = n_b * T // (pp * r)
    hb = r * 32          # out elems per partition per component
    KT = r * 22          # trig elems per half
    bpt = pp * r // T    # batch examples per tile

    # pred rows grouped per tile: partition p holds r consecutive rows
    pred_t = pred.ap().rearrange("(n p q) c -> n p (q c)", p=pp, q=r)

    with (
        tc.tile_pool(name="io", bufs=3) as io_pool,
        tc.tile_pool(name="mid", bufs=2) as mid_pool,
        tc.tile_pool(name="mid1", bufs=2) as mid1_pool,
        tc.tile_pool(name="const", bufs=1) as const_pool,
    ):
        # static gate tile [pp, 3*r*32] (all three components in one scan):
        # 1.0 everywhere, 0.0 at root channels
        gate = const_pool.tile([pp, 3 * hb], f32)
        g4 = gate.rearrange("p (c q j) -> p c q j", c=3, j=32)
        nc.vector.memset(gate, 1.0)
        nc.vector.memset(g4[:, :, :, 0:2], 0.0)       # ch 0, 1
        nc.vector.memset(g4[:, :, :, 6:12:5], 0.0)    # ch 6, 11

        # per-partition constant for Sin biases
        halfpi = const_pool.tile([pp, 1], f32)
        nc.vector.memset(halfpi, PI / 2)

        for i in range(nt):
            raw = io_pool.tile([pp, r * 66], f32)
            nc.sync.dma_start(out=raw, in_=pred_t[i])
            r4 = raw.rearrange("p (q c) -> p q c", c=66)
            # (theta, phi) strided view iterated (pair, rep, joint)
            th_ph_pm = bass.AP(tensor=raw.tensor, offset=raw.offset + 1,
                               ap=[raw.ap[0], [1, 2], [66, r], [3, 22]])

            # ---- trig via half-angle, no range reduction needed ----
            # |x/2| <= 2.85 < pi.  s = Sin(x/2), c = cos(x/2) = Sin(pi/2-|x|/2)
            # sin x = 2sc,  cos x = 1 - 2s^2; scale factors fold downstream.
            # TGH blocks: [s_t | s_p | c_t | c_p] (dense halves)
            tgh = mid1_pool.tile([pp, 4, KT], f32)
            half_out0 = bass.AP(tensor=tgh.tensor, offset=tgh.offset,
                                ap=[tgh.ap[0], [KT, 2], [22, r], [1, 22]])
            half_out2 = bass.AP(tensor=tgh.tensor, offset=tgh.offset + 2 * KT,
                                ap=[tgh.ap[0], [KT, 2], [22, r], [1, 22]])
            nc.scalar.activation(out=half_out0, in_=th_ph_pm, func=ACTF.Sin,
                                 bias=0.0, scale=0.5)
            absx = mid1_pool.tile([pp, 2, r, 22], f32)
            nc.scalar.activation(out=absx[:, :, :, :], in_=th_ph_pm,
                                 func=ACTF.Abs)
            nc.scalar.activation(out=half_out2, in_=absx[:, :, :, :],
                                 func=ACTF.Sin, bias=halfpi[:, 0:1], scale=-0.5)

            # TG2 blocks: [ct~ | cp~ | st~ | sp~]  (run-triple operand order):
            #   st~ = s_t c_t (sin t = 2 st~),  ct~ = 0.5 - s_t^2
            #   sp~ = s_p c_p,                  cp~ = 0.25 - 0.5 s_p^2
            tg2 = mid1_pool.tile([pp, 4, r, 22], f32)
            nc.vector.tensor_tensor(
                out=bass.AP(tensor=tg2.tensor, offset=tg2.offset + 2 * KT,
                            ap=[tg2.ap[0], [KT, 2], [22, r], [1, 22]]),
                in0=tgh[:, 0:2], in1=tgh[:, 2:4], op=ALU.mult)
            sqt = mid1_pool.tile([pp, 2, KT], f32)
            nc.scalar.activation(out=sqt, in_=tgh[:, 0:2], func=ACTF.Square)
            nc.scalar.activation(out=tg2[:, 0], in_=sqt[:, 0].rearrange(
                "p (q k) -> p q k", k=22), func=ACTF.Copy, bias=0.5, scale=-1.0)
            nc.scalar.activation(out=tg2[:, 1], in_=sqt[:, 1].rearrange(
                "p (q k) -> p q k", k=22), func=ACTF.Copy, bias=0.25, scale=-0.5)

            # RR blocks: [rsin4 | 4r | rsin4] so one triple op per run covers
            # all three components.  rd4 = 4r (ACT); rsin4 double-written (POOL)
            rr = mid1_pool.tile([pp, 3, r, 22], f32)
            nc.scalar.activation(out=rr[:, 1], in_=r4[:, :, 0:66:3],
                                 func=ACTF.Copy, bias=0.0, scale=4.0)
            nc.gpsimd.tensor_tensor(
                out=bass.AP(tensor=rr.tensor, offset=rr.offset,
                            ap=[rr.ap[0], [2 * KT, 2], [1, KT]]),
                in0=bass.AP(tensor=rr.tensor, offset=rr.offset + KT,
                            ap=[rr.ap[0], [0, 2], [1, KT]]),
                in1=bass.AP(tensor=tg2.tensor, offset=tg2.offset + 3 * KT,
                            ap=[tg2.ap[0], [0, 2], [1, KT]]),
                op=ALU.mult)

            # ---- W [pp, 3, r, 32]: muls write x0/x1/x2 straight into
            # their output-channel slots (no assembly copies) ----
            w = io_pool.tile([pp, 3, r, 32], f32)
            nc.gpsimd.memset(w[:, :, :, 20:29:8], 0.0)
            nc.gpsimd.memset(w[:, :, :, 23:32:8], 0.0)
            hb32 = r * 32
            # [x0, x1, x2] = [rsin4, 4r, rsin4] * [ct~, cp~, st~]: one DVE
            # op per run GROUP (equal-length runs pair-merged via an extra
            # AP dim with independent per-operand strides)
            for group in RUN_GROUPS:
                k0, ch0, ln = group[0]
                pair_k = [[group[1][0] - k0, 2]] if len(group) == 2 else []
                pair_c = [[group[1][1] - ch0, 2]] if len(group) == 2 else []
                nc.vector.tensor_tensor(
                    out=bass.AP(tensor=w.tensor, offset=w.offset + ch0,
                                ap=[w.ap[0]] + pair_c
                                + [[hb32, 3], [32, r], [1, ln]]),
                    in0=bass.AP(tensor=rr.tensor, offset=rr.offset + k0,
                                ap=[rr.ap[0]] + pair_k
                                + [[KT, 3], [22, r], [1, ln]]),
                    in1=bass.AP(tensor=tg2.tensor, offset=tg2.offset + k0,
                                ap=[tg2.ap[0]] + pair_k
                                + [[KT, 3], [22, r], [1, ln]]),
                    op=ALU.mult)

            # root slots from obs (host-duplicated rows: one row per partition)
            obs_t = mid_pool.tile([pp, 96], f32)
            nc.sync.dma_start(out=obs_t, in_=obs[i * pp : (i + 1) * pp, :])
            nc.scalar.copy(
                out=w[:, :, :, 0:2],
                in_=bass.AP(tensor=obs_t.tensor, offset=obs_t.offset,
                            ap=[obs_t.ap[0], [1, 3], [0, r], [3, 2]]),
            )
            nc.scalar.copy(
                out=w[:, :, :, 6:12:5],
                in_=bass.AP(tensor=obs_t.tensor, offset=obs_t.offset + 18,
                            ap=[obs_t.ap[0], [1, 3], [0, r], [15, 2]]),
            )
            # corrections, reading W itself:
            #   ch16 = -(W14+W15) = -(k2+k3);  ch24 = -(W17..W22) (W20 = 0)
            nc.vector.tensor_reduce(
                out=w[:, :, :, 16:17], in_=w[:, :, :, 14:16],
                axis=mybir.AxisListType.X, op=ALU.add, negate=True)
            nc.vector.tensor_reduce(
                out=w[:, :, :, 24:25], in_=w[:, :, :, 17:23],
                axis=mybir.AxisListType.X, op=ALU.add, negate=True)

            # ---- gated scan, all 3 components fused, out of place ----
            ot = io_pool.tile([pp, 3, hb], f32)
            nc.vector.tensor_tensor_scan(
                out=ot.rearrange("p c f -> p (c f)"), data0=gate,
                data1=w.rearrange("p c q j -> p (c q j)"),
                initial=0.0, op0=ALU.mult, op1=ALU.add)

            # out DMA per component: DRAM [b, c*2048 + t*32 + ch] with
            # b = i*bpt + p // (T//r), t = (p % (T//r))*r + rep
            for c in range(3):
                nc.sync.dma_start(
                    out=bass.AP(
                        tensor=out,
                        offset=(i * bpt) * (3 * T * 32) + c * (T * 32),
                        ap=[[3 * T * 32, bpt], [hb, T // r], [1, hb]],
                    ),
                    in_=ot[:, c],
                )


_CACHE = {}


def _get_nc():
    if "nc" not in _CACHE:
        import concourse.bacc as bacc

        nc = bacc.Bacc("TRN2", target_bir_lowering=False)
        build_kernel(nc, NB, r=KERNEL_R)
        nc.compile()
        _CACHE["nc"] = nc
    return _CACHE["nc"]


def _run(in_maps, **kwargs):
    from concourse.bass_utils import run_bass_kernel_spmd

    nc = _get_nc()
    return run_bass_kernel_spmd(nc, in_maps, core_ids=list(range(N_CORES)), **kwargs)


def _make_in_maps(observed_pose, pred_pose):
    obs_last = np.ascontiguousarray(observed_pose[:, -1, :], dtype=np.float32)
    # one obs row per tile-partition: duplicate each row T//r times
    obs_dup = np.repeat(obs_last, T // KERNEL_R, axis=0)
    pred = np.ascontiguousarray(pred_pose, dtype=np.float32)
    dup = T // KERNEL_R
    in_maps = []
    for c in range(N_CORES):
        in_maps.append(
            {
                "pred": np.ascontiguousarray(
                    pred[c * NB : (c + 1) * NB].reshape(NB * T, 66)
                ),
                "obs": np.ascontiguousarray(obs_dup[c * NB * dup : (c + 1) * NB * dup]),
            }
        )
    return in_maps


def _assemble_out(results):
    outs = []
    for c in range(N_CORES):
        o = results[c]["out"].reshape(NB, 3, T, 32)
        outs.append(o.transpose(0, 2, 3, 1).reshape(NB, T, 96))
    return np.ascontiguousarray(np.concatenate(outs, axis=0), dtype=np.float32)


def kernel(observed_pose, pred_pose):
    res = _run(_make_in_maps(observed_pose, pred_pose))
    return _assemble_out(res.results)


def kernel_traced(observed_pose, pred_pose, trace_cores=None):
    """Like kernel() but returns (output, BassKernelResults) with an NTFF trace."""
    res = _run(
        _make_in_maps(observed_pose, pred_pose),
        trace=True,
        trace_cores=trace_cores or [0],
    )
    return _assemble_out(res.results), res
